# revision 48
# baseline (speedup 1.0000x reference)
"""BarrierNet (MLP 4->512->{128,128}->{2,2} + closed-form QP) on 8 Trainium2 cores.

Data-parallel: batch 262144 sharded 8 x 32768; weights replicated.

Per-core layout: sample s = p*256 + 4t + j (p = SBUF partition, t = 512-sample
tile, j = 0..3). The MLP runs feature-major (batch on the PE free dim):

L1 is fp32r with K=5 (ones row folds b1 into the matmul) writing two
[128,2,512] PSUM tiles; each is drained by a single big ACT/DVE op into h1
as fp8e4. L2 splits by branch: h21 runs four plain fp8 matmuls (1 cyc/row,
output on all 128 partitions -> one 512-col drain), h22 runs fp8 DoubleRow
(2 K-tiles per instruction, 0.5 cyc/row; output restricted to PSUM
partitions 0:64, hidden block m in bank m) with per-m merged drains. h2
stays fp32. L3 is flipped: h2 sample-chunks are the *stationary* operand and
the stacked W3 columns the moving [*, 4] operand, so every matmul writes
[128 samples, 4] sample-major into a per-sp PSUM bank (memset + start=False
accumulation) - no transposes, no staging, no DRAM bounce. The QP (sin/cos
via range-reduced ACT Sin, sigmoid, one reciprocal) runs as [128, 128] ops
per sp half mostly on Pool, reading x_nat which is sample-major by
construction. QP ops are emitted as closure lists drained a few per pipeline
slot so no engine sees a burst that would stall the per-tile critical path.

Engine budget per 512-sample tile (ns): PE 2213 (L1 2048c + L2 3072c + L3
192c), ACT ~2100 (h1-A merged drain, h21 drain, h22 share), DVE ~2100 (h1-B
merged drain, h22 share, QP), Pool ~1000 (QP), DMA ~100.
"""
import numpy as np
from contextlib import ExitStack

import ml_dtypes

import concourse.bass as bass
from concourse import bacc as bacc_mod
import concourse.tile as tile
from concourse import mybir
from concourse.bass_utils import run_bass_kernel_spmd

F32 = mybir.dt.float32
F32R = mybir.dt.float32r
F8 = mybir.dt.float8e4
AF = mybir.ActivationFunctionType
OP = mybir.AluOpType
DR = mybir.MatmulPerfMode.DoubleRow

MAGIC = float(np.float32(1.5 * 2 ** 23))
INV2PI = float(np.float32(1.0 / (2 * np.pi)))
TWOPI = float(np.float32(2 * np.pi))
HALFPI = float(np.float32(np.pi / 2))

N_CORES = 8
NB = 262144
B = NB // N_CORES  # 32768 per core

_CACHE = {}


def _emit(nc, tc, ctx, aps, B):
    (xT5, x_nat, w1r, w2blob, cblob, u_out) = aps
    T = B // 512           # 64 sample tiles per core
    NS = T // 32           # 2 QP half-batches ("sp")

    const = ctx.enter_context(tc.tile_pool(name="const", bufs=1))
    lp = ctx.enter_context(tc.tile_pool(name="lp", bufs=1))
    qp = ctx.enter_context(tc.tile_pool(name="qp", bufs=1))
    ps = ctx.enter_context(tc.tile_pool(name="ps", bufs=1, space="PSUM"))

    # DMA order matters: transfers serialize on the HWDGE/DMA devices, so
    # stage exactly what the pipeline needs first (L1 tile 0, then L2
    # weights, then the remaining x tiles, then QP-only data).
    w1_sb = const.tile([128, 512], F32R, name="w1_sb", tag="w1_sb")
    nc.sync.dma_start(w1_sb[:], w1r[:])
    x4_sb = const.tile([128, T // 4, 512], F32R, name="x4_sb", tag="x4_sb")
    xv = xT5.rearrange("q (tt c r) -> q tt c r", c=4, r=512)
    # X5: tile t lives at partition base 32*(t%4), columns 512*(t//4)..
    nc.sync.dma_start(x4_sb[0:5, :, :], xv[:, :, 0, :])
    # all fp8 weights in one transfer (HWDGE serializes DMAs at ~630ns each)
    w2_sb = const.tile([128, 1024], F8, name="w2_sb", tag="w2_sb")
    nc.sync.dma_start(w2_sb[:], w2blob[:])
    w21_sb = w2_sb[:, 0:512].rearrange("p (k c) -> p k c", k=4)
    w22_sb = w2_sb[:, 512:1024].rearrange(
        "p (kp m i c) -> p kp m i c", kp=2, m=2, i=2)
    for c in range(1, 4):
        nc.sync.dma_start(x4_sb[32 * c:32 * c + 5, :, :], xv[:, :, c, :])
    # all fp32 constants in one transfer; see _prep_core_inputs for layout
    cb_sb = const.tile([128, 276], F32R, name="cb_sb", tag="cb_sb")
    nc.sync.dma_start(cb_sb[:], cblob[:])
    b2_sb = cb_sb[:, 0:2].bitcast(F32)
    b22h_sb = cb_sb[0:64, 2:4].bitcast(F32)
    b3_sb = cb_sb[:, 4:8].bitcast(F32)
    # w3_sb[:, 0, :] = [W31.T | 0] (K=128); w3_sb[0:64, 1+m, :] = [0 | W32.T]
    # for hidden half m (K=64).
    w3_sb = cb_sb[:, 8:20].rearrange("p (i c) -> p i c", i=3)
    b3f_sb = cb_sb[:, 20:276].bitcast(F32).rearrange("p (i c) -> p i c", i=2)
    xn_sb = const.tile([128, NS, 128, 4], F32, name="xn_sb", tag="xn_sb")
    nc.sync.dma_start(xn_sb[:], x_nat[:])
    # broadcast constants for Pool-side affine ops
    cst = {}
    for i, val in enumerate((0.0, 2.0, 5.0, 8.0, 10.0, 16.0, 576.0, -2.0)):
        cst[val] = const.tile([128, 128], F32, name=f"cst{i}", tag=f"cst{i}")
        nc.gpsimd.memset(cst[val][:], val)

    # software-pipelined: window w runs L1(w), L2(w-1), L3(w-2), QP spread
    h1s = {}
    h2s = {}
    ps3s = {}
    geo = {}
    pend = []  # queue of QP op closures, drained a few per slot
    for w in range(T + 4):
        if w < T:
            t = w
            cb = 32 * (t % 4)
            xc = x4_sb[cb:cb + 5, t // 4, :]
            h1 = lp.tile([128, 4, 512], F8, name="h1", tag="h1", bufs=3)
            h1s[t] = h1
            psA = ps.tile([128, 2, 512], F32, name="psA", tag="psA", bufs=1)
            for f in range(2):
                nc.tensor.matmul(
                    psA[:, f, :], w1_sb[cb:cb + 5, 128 * f:128 * (f + 1)],
                    xc, start=True, stop=True, tile_position=(cb, 0))
            nc.scalar.activation(h1[:, 0:2, :], psA[:], AF.Relu)
            psB = ps.tile([128, 2, 512], F32, name="psB", tag="psB", bufs=1)
            for f in range(2):
                nc.tensor.matmul(
                    psB[:, f, :],
                    w1_sb[cb:cb + 5, 128 * (2 + f):128 * (3 + f)],
                    xc, start=True, stop=True, tile_position=(cb, 0))
            nc.scalar.activation(h1[:, 2:4, :], psB[:], AF.Relu)
        if 1 <= w <= T:
            t = w - 1
            h1 = h1s.pop(t)
            # h21 branch: plain fp8 matmuls, out on all 128 partitions.
            ps21 = ps.tile([128, 512], F32, name="ps21", tag="ps21", bufs=1)
            for k in range(4):
                nc.tensor.matmul(ps21[:], w21_sb[:, k, :], h1[:, k, :],
                                 start=(k == 0), stop=(k == 3))
            h21 = lp.tile([128, 512], F32R, name="h21", tag="h21", bufs=2)
            nc.vector.tensor_scalar(h21[:], ps21[:], b2_sb[:, 0:1], 0.0,
                                    op0=OP.add, op1=OP.max)
            # h22 branch: fp8 DoubleRow, out on PSUM partitions 0:64,
            # hidden block m in bank m; h22 stays [64, m, samples].
            ps22 = ps.tile([64, 2, 2, 256], F32, name="ps22", tag="ps22",
                           bufs=1)
            for hh in range(2):
                for m in range(2):
                    for kp in range(2):
                        nc.tensor.matmul(
                            ps22[0:64, m, hh, :],
                            w22_sb[:, kp, m, :, :],
                            h1[:, 2 * kp:2 * kp + 2,
                               256 * hh:256 * hh + 256],
                            start=(hh == 0 and kp == 0), stop=(kp == 1),
                            perf_mode=DR, skip_group_check=True)
            h22 = lp.tile([64, 2, 512], F32R, name="h22", tag="h22", bufs=2)
            h22v = h22[:].rearrange("q m (hh n) -> q m hh n", hh=2)
            for m in range(2):
                nc.vector.tensor_scalar(h22v[:, m, :, :], ps22[0:64, m, :, :],
                                        b22h_sb[:, m:m + 1], 0.0,
                                        op0=OP.add, op1=OP.max)
            h2s[t] = (h21, h22)
        if 2 <= w <= T + 1:
            t = w - 2
            sp, g = divmod(t, 32)
            if g == 0:
                # one PSUM bank, two 64-sample halves; half 1 is zeroed
                # mid-sp so its WAR on the previous sp's reads never stalls
                ps3 = ps.tile([128, 2, 64, 4], F32, name="ps3", tag="ps3",
                              bufs=1)
                ps3s[sp] = ps3
                nc.scalar.memzero(ps3[:, 0, :, :])
                geo[sp] = {}
                pend.extend(_qp_geo_ops(nc, qp, xn_sb, sp, geo[sp], cst))
            ps3 = ps3s[sp]
            if g == 8:
                nc.scalar.memzero(ps3[:, 1, :, :])
            h21, h22 = h2s.pop(t)
            for j in range(4):
                mi = 4 * g + j
                dst = ps3[:, mi // 64, mi % 64, :]
                nc.tensor.matmul(
                    dst, h21[:, 128 * j:128 * (j + 1)],
                    w3_sb[:, 0, :], start=False, stop=False,
                    skip_group_check=True)
                for m in range(2):
                    nc.tensor.matmul(
                        dst, h22[0:64, m, 128 * j:128 * (j + 1)],
                        w3_sb[0:64, 1 + m, :], start=False,
                        stop=(g == 31 and j == 3 and m == 1),
                        skip_group_check=True)
            if g in (15, 31):
                # QP tail chunk: half the sp as soon as its L3 rows landed
                c = g // 16
                s3c = qp.tile([128, 64, 4], F32, name="s3", tag="s3", bufs=2)
                nc.vector.tensor_copy(s3c[:], ps3[:, c, :, :])
                pend.extend(_qp_rest_ops(nc, qp, s3c, b3f_sb, b3_sb, u_out,
                                         sp, c, geo[sp], cst,
                                         fast=(sp == NS - 1 and c == 1)))
                if g == 31:
                    ps3s.pop(sp)
        # drain a few pending QP ops per slot to avoid engine bursts; hold
        # off at the start so an xn_sb-gated op can't block a drain stream
        if w >= 6:
            for _ in range(3):
                if pend:
                    pend.pop(0)()
    while pend:
        pend.pop(0)()


def _qp_tile(nc, qp, name, bufs=1):
    return qp.tile([128, 128], F32, name=name, tag=name, bufs=bufs)


def _qp_geo_ops(nc, qp, xn_sb, sp, out, cst):
    """x-only QP quantities (no MLP dependency) as a list of op closures.

    Almost everything runs on Pool (TensorTensor-only engine) using the
    broadcast const tiles; DVE keeps only the round-trip "magic" ops whose
    intermediate rounding an affine op can't reproduce, plus the
    reciprocal. Results consumed by _qp_rest_ops use bufs=2 tiles.
    """
    r = {}

    def tt(name, a, b, op, bufs=1):
        def f():
            o = _qp_tile(nc, qp, name, bufs=bufs)
            bb = cst[b][:] if isinstance(b, float) else r[b]
            nc.gpsimd.tensor_tensor(o[:], r[a], bb, op=op)
            r[name] = o[:]
            out[name] = o[:]
        return f

    def ts(name, a, s1, op0, s2=None, op1=None, bufs=1):
        def f():
            o = _qp_tile(nc, qp, name, bufs=bufs)
            if s2 is None:
                nc.vector.tensor_scalar(o[:], r[a], s1, None, op0=op0)
            else:
                nc.vector.tensor_scalar(o[:], r[a], s1, s2, op0=op0, op1=op1)
            r[name] = o[:]
            out[name] = o[:]
        return f

    def act(name, a, func):
        def f():
            o = _qp_tile(nc, qp, name)
            nc.scalar.activation(o[:], r[a], func)
            r[name] = o[:]
            out[name] = o[:]
        return f

    def aff(name, a, scale, bias):
        # single-rounding affine -> ACT Copy; NOT valid for the magic
        # round-trip ops, which need the intermediate rounding
        def f():
            o = _qp_tile(nc, qp, name)
            nc.scalar.activation(o[:], r[a], AF.Copy, bias=bias, scale=scale)
            r[name] = o[:]
            out[name] = o[:]
        return f

    r["PX"] = xn_sb[:, sp, :, 0]
    r["PY"] = xn_sb[:, sp, :, 1]
    r["TH"] = xn_sb[:, sp, :, 2]
    r["VV"] = xn_sb[:, sp, :, 3]

    # Phase-ordered: ops land in each engine's in-order stream only after
    # their cross-engine inputs are (nearly) ready, so the ACT/DVE streams
    # that also carry the critical PSUM drains never block on the Pool chain.
    return [
        # instant ACT/DVE ops (inputs straight from xn_sb) and the sin-free
        # Pool chain
        aff("f1", "TH", INV2PI, MAGIC),
        aff("c1", "TH", INV2PI, 0.25),
        ts("f2", "f1", MAGIC, OP.subtract, TWOPI, OP.mult),
        ts("c2", "c1", MAGIC, OP.add, MAGIC, OP.subtract),
        aff("c3", "c2", TWOPI, -HALFPI),
        tt("thr", "TH", "f2", OP.subtract),
        tt("thc", "TH", "c3", OP.subtract),
        tt("dxa", "PX", 10.0, OP.mult),
        tt("dx", "dxa", 10.0, OP.add),
        tt("dya", "PY", 10.0, OP.mult),
        tt("dy", "dya", 5.0, OP.add),
        tt("va", "VV", 2.0, OP.mult),
        tt("v", "va", 5.0, OP.add),
        tt("dx2", "dx", "dx", OP.mult),
        tt("dy2", "dy", "dy", OP.mult),
        tt("bar", "dx2", "dy2", OP.add),
        tt("bar16a", "bar", 16.0, OP.mult),
        tt("bar16", "bar16a", 576.0, OP.subtract, bufs=2),
        tt("v2", "v", "v", OP.mult),
        tt("v22", "v2", 2.0, OP.mult, bufs=2),
        # ACT sins pop here, their thr/thc inputs long since computed
        act("st", "thr", AF.Sin),
        act("ct", "thc", AF.Sin),
        tt("vst", "v", "st", OP.mult),
        tt("vct", "v", "ct", OP.mult),
        tt("a3", "dx", "vct", OP.mult),
        tt("a4", "dy", "vst", OP.mult),
        tt("a5", "a3", "a4", OP.add),
        tt("bdot4", "a5", 8.0, OP.mult, bufs=2),
        tt("g1a", "dx", "vst", OP.mult),
        tt("g1b", "dy", "vct", OP.mult),
        tt("G1p", "g1a", "g1b", OP.subtract, bufs=2),
        tt("g2a", "dx", "ct", OP.mult),
        tt("g2b", "dy", "st", OP.mult),
        tt("G2pp", "g2a", "g2b", OP.add, bufs=2),
        tt("q1", "G1p", "G1p", OP.mult),
        tt("q2", "G2pp", "G2pp", OP.mult),
        tt("q3", "q1", "q2", OP.add),
        # trailing DVE pair pops ~12 slots in; q3 is ready by then
        ts("ggc", "q3", 4.0, OP.mult, 1e-12, OP.max),
        _recip(nc, qp, r, out),
    ]


def _recip(nc, qp, r, out):
    def f():
        o = _qp_tile(nc, qp, "rec", bufs=2)
        nc.vector.reciprocal(o[:], r["ggc"])
        out["rec"] = o[:]
    return f


def _qp_rest_ops(nc, qp, s3c, b3f_sb, b3_sb, u_out, sp, c, g, cst,
                 fast=False):
    """QP epilogue for sample half-chunk c of sp (columns 64c..64c+64).

    Normally the whole chain runs on Pool (TensorTensor pairs against const
    tiles) to keep ACT/DVE free for the per-tile drains. fast=True (final
    chunk) runs it on DVE with fused scalar_tensor_tensor ops instead:
    nothing else needs DVE then, and its ops are ~3x quicker, shortening the
    wind-down tail.
    """
    r = {}
    csl = slice(64 * c, 64 * c + 64)
    chain_eng = nc.vector if fast else nc.gpsimd

    def get(k):
        if k in r:
            return r[k]
        if isinstance(k, float):
            return cst[k][:, 0:64]
        return g[k][:, csl]

    def tt(name, a, b, op, eng=None):
        def f():
            o = qp.tile([128, 64], F32, name=name, tag="r_" + name, bufs=1)
            (eng or chain_eng).tensor_tensor(o[:], get(a), get(b), op=op)
            r[name] = o[:]
        return f

    def stt(name, a, s, b, op0, op1):
        def f():
            o = qp.tile([128, 64], F32, name=name, tag="r_" + name, bufs=1)
            nc.vector.scalar_tensor_tensor(o[:], get(a), s, get(b),
                                           op0=op0, op1=op1)
            r[name] = o[:]
        return f

    def sig(name, src, bcol):
        def f():
            o = qp.tile([128, 64], F32, name=name, tag="r_" + name, bufs=1)
            nc.scalar.activation(o[:], src, AF.Sigmoid,
                                 bias=b3_sb[:, bcol:bcol + 1])
            r[name] = o[:]
        return f

    def addb(name, src, i):
        def f():
            o = qp.tile([128, 64], F32, name=name, tag="r_" + name, bufs=1)
            chain_eng.tensor_tensor(o[:], src, b3f_sb[:, i, csl], op=OP.add)
            r[name] = o[:]
        return f

    X31A = s3c[:, :, 0]
    X31B = s3c[:, :, 1]
    Z32A = s3c[:, :, 2]
    Z32B = s3c[:, :, 3]

    def emit_u():
        u_sb = qp.tile([128, 64, 2], F32, name="u_sb", tag="u_sb", bufs=2)
        chain_eng.tensor_tensor(u_sb[:, :, 0], r["z1"], r["xa"],
                                op=OP.subtract)
        (nc.vector if fast else nc.gpsimd).tensor_tensor(
            u_sb[:, :, 1], r["z2"], r["xb"], op=OP.subtract)
        nc.sync.dma_start(u_out[:, sp, csl, :], u_sb[:])

    def relu(name, a):
        def f():
            o = qp.tile([128, 64], F32, name=name, tag="r_" + name, bufs=1)
            nc.scalar.activation(o[:], get(a), AF.Relu)
            r[name] = o[:]
        return f

    ops = [
        sig("sa", Z32A, 2),
        addb("xa", X31A, 0),
        sig("sb_", Z32B, 3),
        addb("xb", X31B, 1),
        tt("ssum", "sa", "sb_", OP.add),
        tt("sprod", "sa", "sb_", OP.mult),
        tt("m1", "ssum", "bdot4", OP.mult),
        tt("m2", "sprod", "bar16", OP.mult),
        tt("n1", "G1p", "xa", OP.mult),
        tt("m3", "m1", "m2", OP.add),
        tt("n2", "G2pp", "xb", OP.mult),
        tt("h", "v22", "m3", OP.add),
        tt("n3", "n1", "n2", OP.subtract),
    ]
    if fast:
        ops += [
            stt("num", "n3", -2.0, "h", OP.mult, OP.subtract),
            stt("lam", "num", 0.0, "rec", OP.max, OP.mult),
            stt("z1", "lam", -2.0, "G1p", OP.mult, OP.mult),
            stt("z2", "lam", 2.0, "G2pp", OP.mult, OP.mult),
        ]
    else:
        ops += [
            tt("nn", "n3", -2.0, OP.mult),
            tt("num", "nn", "h", OP.subtract),
            relu("lam0", "num"),
            tt("lam", "lam0", "rec", OP.mult),
            tt("z1a", "lam", "G1p", OP.mult),
            tt("z1", "z1a", -2.0, OP.mult),
            tt("z2a", "lam", "G2pp", OP.mult),
            tt("z2", "z2a", 2.0, OP.mult),
        ]
    return ops + [emit_u]


def _build_kernel(n_cores, B):
    nc = bacc_mod.Bacc("TRN2", target_bir_lowering=False, debug=False,
                       num_devices=n_cores)
    NS = (B // 512) // 32
    xT5 = nc.dram_tensor("xT5", [5, B], F32R, kind="ExternalInput").ap()
    x_nat = nc.dram_tensor("x_nat", [128, NS, 128, 4], F32,
                           kind="ExternalInput").ap()
    w1r = nc.dram_tensor("w1r", [128, 512], F32R, kind="ExternalInput").ap()
    w2blob = nc.dram_tensor("w2blob", [128, 1024], F8,
                            kind="ExternalInput").ap()
    cblob = nc.dram_tensor("cblob", [128, 276], F32R,
                           kind="ExternalInput").ap()
    u_out = nc.dram_tensor("u_out", [128, NS, 128, 2], F32,
                           kind="ExternalOutput").ap()
    aps = (xT5, x_nat, w1r, w2blob, cblob, u_out)
    with tile.TileContext(nc) as tc:
        with ExitStack() as ctx:
            _emit(nc, tc, ctx, aps, B)
    nc.compile()
    return nc


def _prep_core_inputs(x_shard, W1, b1, W21, b21, W22, b22, W31, b31, W32, b32):
    Bc = x_shard.shape[0]
    T = Bc // 512
    xs = np.ascontiguousarray(x_shard, dtype=np.float32).reshape(
        128, T, 4, 4)  # [p, t, j, feat]
    xT5 = np.empty((5, Bc), dtype=np.float32)
    xT5[:4] = xs.transpose(3, 1, 2, 0).reshape(4, Bc)
    xT5[4] = 1.0

    w1r = np.zeros((128, 512), dtype=np.float32)
    w1e = np.concatenate([W1.T, b1[None, :]], axis=0)
    for c in range(4):
        w1r[32 * c:32 * c + 5, :] = w1e

    # h21 branch: plain fp8 stationary [k, kb, c] = W21[c, kb*128 + k]
    w21s = W21.T.reshape(4, 128, 128).transpose(1, 0, 2)

    # h22 branch DoubleRow stationary: [k, kp, m, i, c] =
    #   W22[m*64 + c, (2*kp + i)*128 + k]
    w22s = W22.reshape(2, 64, 4, 128).transpose(3, 2, 0, 1)  # k, kb, m, c
    w22s = w22s.reshape(128, 2, 2, 2, 64)  # k, kp, i, m, c
    w22s = w22s.transpose(0, 1, 3, 2, 4)  # k, kp, m, i, c
    w2blob = np.concatenate(
        [w21s.reshape(128, 512), w22s.reshape(128, 512)],
        axis=1).astype(ml_dtypes.float8_e4m3)

    # cblob: [b2s(2) | b22h(2) | b3bc(4) | w3s(12) | b3f(256)]
    cblob = np.zeros((128, 276), dtype=np.float32)
    cblob[:, 0:2] = np.stack([b21, b22], axis=1)
    cblob[0:64, 2:4] = b22.reshape(2, 64).T
    cblob[:, 4:8] = np.concatenate([b31, b32])[None, :]
    w3s = np.zeros((128, 3, 4), dtype=np.float32)
    w3s[:, 0, 0:2] = W31.T
    w3s[0:64, 1, 2:4] = W32.T[0:64]
    w3s[0:64, 2, 2:4] = W32.T[64:128]
    cblob[:, 8:20] = w3s.reshape(128, 12)
    cblob[:, 20:148] = b31[0]
    cblob[:, 148:276] = b31[1]

    return {
        "xT5": xT5,
        "x_nat": np.ascontiguousarray(x_shard, dtype=np.float32).reshape(
            128, Bc // (128 * 128), 128, 4),
        "w1r": w1r,
        "w2blob": w2blob,
        "cblob": cblob,
    }


def kernel(x, W1, b1, W21, b21, W22, b22, W31, b31, W32, b32, sgn=None):
    x = np.asarray(x, dtype=np.float32)
    args = [np.asarray(a, dtype=np.float32)
            for a in (W1, b1, W21, b21, W22, b22, W31, b31, W32, b32)]

    if "nc" not in _CACHE:
        _CACHE["nc"] = _build_kernel(N_CORES, B)
    nc = _CACHE["nc"]

    in_maps = []
    for c in range(N_CORES):
        shard = x[c * B:(c + 1) * B]
        in_maps.append(_prep_core_inputs(shard, *args))

    res = run_bass_kernel_spmd(nc, in_maps, core_ids=list(range(N_CORES)))
    out = np.empty((NB, 2), dtype=np.float32)
    for c in range(N_CORES):
        out[c * B:(c + 1) * B] = res.results[c]["u_out"].reshape(B, 2)
    return out
